# revision 9
# baseline (speedup 1.0000x reference)
"""Capsule dynamic-routing kernel for Trainium2, 8 NeuronCores.

Full inputs in, full output out. Sharding: n_in (2048) split 8 ways; every core
keeps the whole batch. The only cross-core traffic is an AllReduce of the
routing sum s[b, caps_n, caps_dim] (256 KB) once per routing iteration.

Per core, per routing round, u_hat is recomputed on the PE with a
block-diagonal-x stationary so each matmul runs with K=128/N=512 instead of
K=16/N=32 (the naive per-i batched matvec shape).
"""
import sys

if "/opt/trn_rl_repo" not in sys.path:
    sys.path.insert(0, "/opt/trn_rl_repo")

import numpy as np
import ml_dtypes

import concourse.bass as bass
import concourse.mybir as mybir
import concourse.tile as tile
from concourse import bacc, bass_utils

F32 = mybir.dt.float32
BF16 = mybir.dt.bfloat16
AX = mybir.AxisListType
OP = mybir.AluOpType
ACTF = mybir.ActivationFunctionType

N_CORES = 8
B = 32          # batch
NI_FULL = 2048  # n_in total
NI = NI_FULL // N_CORES  # 256 per core
KN = 64         # caps_n
D = 32          # caps_dim
L = 16          # d_in
KD = KN * D     # 2048
NIB = NI // 8   # 32 i-blocks of 8 i's per core
EPS = 1e-7
ROUTINGS = 3

_CACHE = {}


def _build_nc(sim=False):
    nc = bacc.Bacc("TRN2", num_devices=1 if sim else N_CORES)

    wr_d = nc.dram_tensor("wr", [NIB, 128, KD], BF16, kind="ExternalInput")
    sx_d = nc.dram_tensor("sx", [2 * NIB, 128, 128], BF16, kind="ExternalInput")
    xt_d = nc.dram_tensor("xt", [NIB, 128, B], BF16, kind="ExternalInput")
    bs_d = nc.dram_tensor("bs", [2, 128, B], BF16, kind="ExternalInput")
    v_out_d = nc.dram_tensor("v_out", [B, KD], F32, kind="ExternalOutput")

    cc_in = [nc.dram_tensor(f"cc_in{r}", [B, KD], F32, kind="Internal")
             for r in range(ROUTINGS)]
    cc_out = [nc.dram_tensor(f"cc_out{r}", [B, KD], F32, kind="Internal",
                             addr_space="Shared")
              for r in range(ROUTINGS)]

    with tile.TileContext(nc) as tc:
        with tc.tile_pool(name="singles", bufs=1) as singles, \
             tc.tile_pool(name="wstream", bufs=3) as wstream, \
             tc.tile_pool(name="upool", bufs=2, space="PSUM") as upool, \
             tc.tile_pool(name="spool", bufs=1, space="PSUM") as spool, \
             tc.tile_pool(name="usb", bufs=3) as usbp, \
             tc.tile_pool(name="pp", bufs=2) as pp, \
             tc.tile_pool(name="cup", bufs=2) as cup, \
             tc.tile_pool(name="small", bufs=4) as small:

            # ---- resident tensors ----
            sx_sb = singles.tile([128, 2 * NIB * 128], BF16, name="sx_sb")
            xt_sb = singles.tile([128, NIB * B], BF16, name="xt_sb")
            bs_sb = singles.tile([128, 2 * B], BF16, name="bs_sb")
            b_state = singles.tile([128, 64 * KN], F32, name="b_state")
            vrep = singles.tile([128, 2 * KD], BF16, name="vrep")
            s_sb = singles.tile([B, KD], F32, name="s_sb")
            sr_sb = singles.tile([B, KD], F32, name="sr_sb")
            sq_sb = singles.tile([B, KD], F32, name="sq_sb")
            n2_sb = singles.tile([B, KN], F32, name="n2_sb")
            rt_sb = singles.tile([B, KN], F32, name="rt_sb")
            rc2_sb = singles.tile([B, KN], F32, name="rc2_sb")
            f_sb = singles.tile([B, KN], F32, name="f_sb")
            v_f32 = singles.tile([B, KD], F32, name="v_f32")
            vbf = singles.tile([B, KD], BF16, name="vbf")

            for _t in range(2 * NIB):
                nc.gpsimd.dma_start(
                    sx_sb[:, _t * 128:(_t + 1) * 128], sx_d.ap()[_t])
            for _t in range(NIB):
                nc.gpsimd.dma_start(
                    xt_sb[:, _t * B:(_t + 1) * B], xt_d.ap()[_t])
            for _t in range(2):
                nc.gpsimd.dma_start(
                    bs_sb[:, _t * B:(_t + 1) * B], bs_d.ap()[_t])

            def sxt(t):
                return sx_sb[:, t * 128:(t + 1) * 128]

            def xtt(ib):
                return xt_sb[:, ib * B:(ib + 1) * B]

            def bst(h):
                return bs_sb[:, h * B:(h + 1) * B]

            s_ps = spool.tile([B, KD], F32, name="s_ps")

            def allreduce(r):
                if sim:
                    nc.sync.dma_start(cc_out[r].ap(), cc_in[r].ap())
                else:
                    nc.gpsimd.collective_compute(
                        "AllReduce", OP.add,
                        replica_groups=[list(range(N_CORES))],
                        ins=[cc_in[r].ap()], outs=[cc_out[r].ap()])

            def squash_and_bcast(r, alpha, last):
                """cc_out[r] -> v; write vrep (if not last) or v_out (if last).
                v = squash(alpha * s); folded: n2 = a^2*ss + EPS,
                f = alpha*sqrt(n2)/(1+n2), v = s*f (elementwise, f bcast on d)."""
                nc.sync.dma_start(sr_sb[:], cc_out[r].ap())
                nc.vector.tensor_tensor(sq_sb[:], sr_sb[:], sr_sb[:], OP.mult)
                nc.vector.tensor_reduce(
                    n2_sb[:], sq_sb[:].rearrange("b (k d) -> b k d", k=KN),
                    AX.X, OP.add)
                nc.vector.tensor_scalar(
                    n2_sb[:], n2_sb[:], alpha * alpha, EPS,
                    OP.mult, OP.add)
                nc.scalar.activation(rt_sb[:], n2_sb[:], ACTF.Sqrt)
                nc.vector.tensor_scalar_add(rc2_sb[:], n2_sb[:], 1.0)
                nc.vector.reciprocal(rc2_sb[:], rc2_sb[:])
                nc.vector.tensor_tensor(f_sb[:], rt_sb[:], rc2_sb[:], OP.mult)
                out_ap = v_f32[:]
                nc.vector.scalar_tensor_tensor(
                    out_ap, sr_sb[:], alpha,
                    f_sb[:].unsqueeze(2).broadcast_to((B, KN, D)),
                    op0=OP.mult, op1=OP.mult)
                if last:
                    nc.sync.dma_start(v_out_d.ap(), v_f32[:])
                else:
                    nc.scalar.copy(vbf[:], v_f32[:])
                    for h in range(2):
                        for j in range(8):
                            nc.sync.dma_start(
                                vrep[j * 16:(j + 1) * 16,
                                     h * KD:(h + 1) * KD],
                                vbf[h * 16:(h + 1) * 16, :])

            # ================= round 0: s0 = XT^T @ W, c uniform =========
            for ib in range(NIB):
                w = wstream.tile([128, KD], BF16, name="w", tag="w")
                nc.sync.dma_start(w[:], wr_d.ap()[ib])
                for j in range(4):
                    nc.tensor.matmul(
                        s_ps[:, j * 512:(j + 1) * 512],
                        xtt(ib), w[:, j * 512:(j + 1) * 512],
                        start=(ib == 0), stop=(ib == NIB - 1))
            nc.scalar.copy(s_sb[:], s_ps[:])
            nc.sync.dma_start(cc_in[0].ap(), s_sb[:])
            allreduce(0)
            squash_and_bcast(0, 1.0 / KN, last=False)

            # ================= rounds 1, 2 ===============================
            for r in (1, 2):
                for ib in range(NIB):
                    w = wstream.tile([128, KD], BF16, name="w", tag="w")
                    nc.sync.dma_start(w[:], wr_d.ap()[ib])
                    for h in range(2):
                        t = ib * 2 + h
                        u0 = upool.tile([128, 1024], F32, name="u0", tag="u")
                        u1 = upool.tile([128, 1024], F32, name="u1", tag="u")
                        nc.tensor.matmul(u0[:, :512], sxt(t), w[:, :512],
                                         start=True, stop=True)
                        nc.tensor.matmul(u0[:, 512:], sxt(t), w[:, 512:1024],
                                         start=True, stop=True)
                        nc.tensor.matmul(u1[:, :512], sxt(t), w[:, 1024:1536],
                                         start=True, stop=True)
                        nc.tensor.matmul(u1[:, 512:], sxt(t), w[:, 1536:],
                                         start=True, stop=True)
                        usb = usbp.tile([128, KD], BF16, name="usb")
                        nc.scalar.copy(usb[:, :1024], u0[:])
                        nc.scalar.copy(usb[:, 1024:], u1[:])
                        # agreement: P = u_hat * v ; A = sum_d P
                        p_t = pp.tile([128, KD], BF16, name="p_t")
                        nc.vector.tensor_tensor(
                            p_t[:], usb[:], vrep[:, h * KD:(h + 1) * KD],
                            OP.mult)
                        bsl = b_state[:, t * KN:(t + 1) * KN]
                        if r == 1:
                            nc.vector.tensor_reduce(
                                bsl, p_t[:].rearrange("p (k d) -> p k d", k=KN),
                                AX.X, OP.add)
                        else:
                            a2 = small.tile([128, KN], F32, name="a2")
                            nc.vector.tensor_reduce(
                                a2[:], p_t[:].rearrange("p (k d) -> p k d", k=KN),
                                AX.X, OP.add)
                            nc.vector.tensor_tensor(bsl, bsl, a2[:], OP.add)
                        # c = softmax_k(b)  (no max-sub; |b| < ~16)
                        e_t = small.tile([128, KN], F32, name="e_t")
                        nc.scalar.activation(e_t[:], bsl, ACTF.Exp)
                        rs = small.tile([128, 1], F32, name="rs")
                        nc.vector.tensor_reduce(rs[:], e_t[:], AX.X, OP.add)
                        rc = small.tile([128, 1], F32, name="rc")
                        nc.vector.reciprocal(rc[:], rs[:])
                        cbf = small.tile([128, KN], BF16, name="cbf")
                        nc.scalar.activation(cbf[:], e_t[:], ACTF.Copy,
                                             scale=rc[:])
                        # cU = c * u_hat ; accumulate s += Bsel^T @ cU
                        cu = cup.tile([128, KD], BF16, name="cu")
                        nc.gpsimd.tensor_tensor(
                            cu[:], usb[:],
                            cbf[:].unsqueeze(2).broadcast_to((128, KN, D)),
                            OP.mult)
                        for j in range(4):
                            nc.tensor.matmul(
                                s_ps[:, j * 512:(j + 1) * 512],
                                bst(h), cu[:, j * 512:(j + 1) * 512],
                                start=(t == 0), stop=(t == 2 * NIB - 1))
                nc.scalar.copy(s_sb[:], s_ps[:])
                nc.sync.dma_start(cc_in[r].ap(), s_sb[:])
                allreduce(r)
                squash_and_bcast(r, 1.0, last=(r == ROUTINGS - 1))

    nc.compile()
    return nc


def _prep_inputs(x, W):
    """Per-core host-side input prep. x [B, 2048, 16] f32, W [2048,64,32,16]."""
    bf = ml_dtypes.bfloat16
    in_maps = []
    for c in range(N_CORES):
        sl = slice(c * NI, (c + 1) * NI)
        Wc = W[sl]                                   # [256, 64, 32, 16]
        wr = np.ascontiguousarray(
            Wc.transpose(0, 3, 1, 2).reshape(NIB, 128, KD)).astype(bf)
        xc = x[:, sl, :]                             # [32, 256, 16]
        xt = np.ascontiguousarray(
            xc.transpose(1, 2, 0).reshape(NIB, 8, L, B)
              .reshape(NIB, 128, B)).astype(bf)
        # SX[(ib h), i8*16+l, i8*16+bl] = x[h*16+bl, ib*8+i8, l]
        sx = np.zeros((2 * NIB, 128, 128), np.float32)
        t5 = xc.reshape(2, 16, NIB, 8, L).transpose(2, 0, 3, 4, 1)
        # t5: [ib, h, i8, l, bl]
        for i8 in range(8):
            sx.reshape(NIB, 2, 128, 128)[
                :, :, i8 * 16:(i8 + 1) * 16, i8 * 16:(i8 + 1) * 16] = \
                t5[:, :, i8]
        sx = sx.astype(bf)
        bsm = np.zeros((2, 128, B), np.float32)
        for h in range(2):
            for i8 in range(8):
                for bl in range(16):
                    bsm[h, i8 * 16 + bl, h * 16 + bl] = 1.0
        bsm = bsm.astype(bf)
        in_maps.append({"wr": wr, "sx": sx, "xt": xt, "bs": bsm})
    return in_maps


def kernel(x, W):
    x = np.asarray(x, dtype=np.float32)
    W = np.asarray(W, dtype=np.float32)
    if "nc" not in _CACHE:
        _CACHE["nc"] = _build_nc()
    nc = _CACHE["nc"]
    in_maps = _prep_inputs(x, W)
    import time as _time
    t0 = _time.time()
    res = bass_utils.run_bass_kernel_spmd(
        nc, in_maps, core_ids=list(range(N_CORES)))
    _CACHE["exec_wall_ns"] = int((_time.time() - t0) * 1e9)
    v = res.results[0]["v_out"].reshape(B, KN, D).astype(np.float32)
    return v


# revision 22
# speedup vs baseline: 1.3051x; 1.3051x over previous
"""Capsule dynamic-routing kernel for Trainium2, 8 NeuronCores.

Full inputs in, full output out. Sharding: n_in (2048) split 8 ways; every core
keeps the whole batch. The only cross-core traffic is an AllReduce of the
routing sum s[b, caps_n, caps_dim] (256 KB) once per routing iteration.

Per core, per routing round, u_hat is recomputed on the PE with a
block-diagonal-x stationary so each matmul runs with K=128/N=512 instead of
K=16/N=32 (the naive per-i batched matvec shape).
"""
import sys

if "/opt/trn_rl_repo" not in sys.path:
    sys.path.insert(0, "/opt/trn_rl_repo")

import numpy as np
import ml_dtypes

import concourse.bass as bass
import concourse.mybir as mybir
import concourse.tile as tile
from concourse import bacc, bass_utils

F32 = mybir.dt.float32
BF16 = mybir.dt.bfloat16
AX = mybir.AxisListType
OP = mybir.AluOpType
ACTF = mybir.ActivationFunctionType

N_CORES = 8
B = 32          # batch
NI_FULL = 2048  # n_in total
NI = NI_FULL // N_CORES  # 256 per core
KN = 64         # caps_n
D = 32          # caps_dim
L = 16          # d_in
KD = KN * D     # 2048
NIB = NI // 8   # 32 i-blocks of 8 i's per core
EPS = 1e-7
ROUTINGS = 3

_CACHE = {}


def _build_nc(sim=False):
    import os as _os
    only_r0 = _os.environ.get("K_ONLY_R0") == "1"
    no_cc = _os.environ.get("K_NO_CC") == "1"
    nc = bacc.Bacc("TRN2", num_devices=1 if sim else N_CORES)

    wr_d = nc.dram_tensor("wr", [NIB, 128, KD], BF16, kind="ExternalInput")
    sx_d = nc.dram_tensor("sx", [2 * NIB, 128, 128], BF16, kind="ExternalInput")
    xt_d = nc.dram_tensor("xt", [NIB, 128, B], BF16, kind="ExternalInput")
    bs_d = nc.dram_tensor("bs", [2, 128, B], BF16, kind="ExternalInput")
    v_out_d = nc.dram_tensor("v_out", [B, KD], F32, kind="ExternalOutput")

    cc_in = [nc.dram_tensor(f"cc_in{r}", [B, KD], F32, kind="Internal")
             for r in range(ROUTINGS)]
    cc_out = [nc.dram_tensor(f"cc_out{r}", [B, KD], F32, kind="Internal",
                             addr_space="Shared")
              for r in range(ROUTINGS)]

    with tile.TileContext(nc) as tc:
        with tc.tile_pool(name="singles", bufs=1) as singles, \
             tc.tile_pool(name="wstream", bufs=4) as wstream, \
             tc.tile_pool(name="upool", bufs=2, space="PSUM") as upool, \
             tc.tile_pool(name="spool", bufs=1, space="PSUM") as spool, \
             tc.tile_pool(name="usb", bufs=6) as usbp, \
             tc.tile_pool(name="pp", bufs=4) as pp, \
             tc.tile_pool(name="cup", bufs=4) as cup, \
             tc.tile_pool(name="small", bufs=8) as small:

            # ---- resident tensors ----
            sx_sb = singles.tile([128, 2 * NIB * 128], BF16, name="sx_sb")
            xt_sb = singles.tile([128, NIB * B], BF16, name="xt_sb")
            bs_sb = singles.tile([128, 2 * B], BF16, name="bs_sb")
            b_state = singles.tile([128, 64 * KN], F32, name="b_state")
            vrep = singles.tile([128, 2 * KD], BF16, name="vrep")
            s_sb = singles.tile([B, KD], F32, name="s_sb")
            sr_sb = singles.tile([B, KD], F32, name="sr_sb")
            sq_sb = singles.tile([B, KD], F32, name="sq_sb")
            n2_sb = singles.tile([B, KN], F32, name="n2_sb")
            rt_sb = singles.tile([B, KN], F32, name="rt_sb")
            rc2_sb = singles.tile([B, KN], F32, name="rc2_sb")
            f_sb = singles.tile([B, KN], F32, name="f_sb")
            v_f32 = singles.tile([B, KD], F32, name="v_f32")
            vbf = singles.tile([B, KD], BF16, name="vbf")

            for _t in range(2 * NIB):
                nc.gpsimd.dma_start(
                    sx_sb[:, _t * 128:(_t + 1) * 128], sx_d.ap()[_t])
            for _t in range(NIB):
                nc.gpsimd.dma_start(
                    xt_sb[:, _t * B:(_t + 1) * B], xt_d.ap()[_t])
            for _t in range(2):
                nc.gpsimd.dma_start(
                    bs_sb[:, _t * B:(_t + 1) * B], bs_d.ap()[_t])

            def sxt(t):
                return sx_sb[:, t * 128:(t + 1) * 128]

            def xtt(ib):
                return xt_sb[:, ib * B:(ib + 1) * B]

            def bst(h):
                return bs_sb[:, h * B:(h + 1) * B]

            s_ps = spool.tile([B, KD], F32, name="s_ps")

            def allreduce(r):
                if sim:
                    nc.sync.dma_start(cc_out[r].ap(), cc_in[r].ap())
                else:
                    nc.gpsimd.collective_compute(
                        "AllReduce", OP.add,
                        replica_groups=[list(range(N_CORES))],
                        ins=[cc_in[r].ap()], outs=[cc_out[r].ap()])

            def squash_and_bcast(r, alpha, last):
                """cc_out[r] -> v; write vrep (if not last) or v_out (if last).
                v = squash(alpha * s); folded: n2 = a^2*ss + EPS,
                f = alpha*sqrt(n2)/(1+n2), v = s*f (elementwise, f bcast on d)."""
                nc.sync.dma_start(sr_sb[:], cc_out[r].ap())
                nc.vector.tensor_tensor(sq_sb[:], sr_sb[:], sr_sb[:], OP.mult)
                nc.vector.tensor_reduce(
                    n2_sb[:], sq_sb[:].rearrange("b (k d) -> b k d", k=KN),
                    AX.X, OP.add)
                nc.vector.tensor_scalar(
                    n2_sb[:], n2_sb[:], alpha * alpha, EPS,
                    OP.mult, OP.add)
                nc.scalar.activation(rt_sb[:], n2_sb[:], ACTF.Sqrt)
                nc.vector.tensor_scalar_add(rc2_sb[:], n2_sb[:], 1.0)
                nc.vector.reciprocal(rc2_sb[:], rc2_sb[:])
                nc.vector.tensor_tensor(f_sb[:], rt_sb[:], rc2_sb[:], OP.mult)
                out_ap = v_f32[:]
                nc.vector.scalar_tensor_tensor(
                    out_ap, sr_sb[:], alpha,
                    f_sb[:].unsqueeze(2).broadcast_to((B, KN, D)),
                    op0=OP.mult, op1=OP.mult)
                if last:
                    nc.sync.dma_start(v_out_d.ap(), v_f32[:])
                else:
                    nc.scalar.copy(vbf[:], v_f32[:])
                    for h in range(2):
                        for j in range(8):
                            nc.sync.dma_start(
                                vrep[j * 16:(j + 1) * 16,
                                     h * KD:(h + 1) * KD],
                                vbf[h * 16:(h + 1) * 16, :])

            # ================= round 0: s0 = XT^T @ W, c uniform =========
            for ib in range(NIB):
                w = wstream.tile([128, KD], BF16, name="w", tag="w")
                nc.sync.dma_start(w[:], wr_d.ap()[ib])
                for j in range(4):
                    nc.tensor.matmul(
                        s_ps[:, j * 512:(j + 1) * 512],
                        xtt(ib), w[:, j * 512:(j + 1) * 512],
                        start=(ib == 0), stop=(ib == NIB - 1))
            nc.scalar.copy(s_sb[:], s_ps[:])
            nc.sync.dma_start(cc_in[0].ap(), s_sb[:])
            if not no_cc:
                allreduce(0)
                squash_and_bcast(0, 1.0 / KN, last=False)
            else:
                nc.scalar.copy(vbf[:], s_sb[:])
                for h in range(2):
                    for j in range(8):
                        nc.sync.dma_start(
                            vrep[j * 16:(j + 1) * 16, h * KD:(h + 1) * KD],
                            vbf[h * 16:(h + 1) * 16, :])
            if only_r0:
                nc.sync.dma_start(v_out_d.ap(), s_sb[:])

            # ================= rounds 1, 2 ===============================
            for r in () if only_r0 else (1, 2):
                for ib in range(NIB):
                    w = wstream.tile([128, KD], BF16, name="w", tag="w")
                    nc.sync.dma_start(w[:], wr_d.ap()[ib])
                    for h in range(2):
                        t = ib * 2 + h
                        u0 = upool.tile([128, 1024], F32, name="u0", tag="u")
                        u1 = upool.tile([128, 1024], F32, name="u1", tag="u")
                        nc.tensor.matmul(u0[:, :512], sxt(t), w[:, :512],
                                         start=True, stop=True)
                        nc.tensor.matmul(u0[:, 512:], sxt(t), w[:, 512:1024],
                                         start=True, stop=True)
                        nc.tensor.matmul(u1[:, :512], sxt(t), w[:, 1024:1536],
                                         start=True, stop=True)
                        nc.tensor.matmul(u1[:, 512:], sxt(t), w[:, 1536:],
                                         start=True, stop=True)
                        usb = usbp.tile([128, KD], BF16, name="usb")
                        nc.scalar.copy(usb[:, :1024], u0[:])
                        nc.scalar.copy(usb[:, 1024:], u1[:])
                        # agreement: P = u_hat * v ; A = sum_d P
                        p_t = pp.tile([128, KD], BF16, name="p_t")
                        nc.vector.tensor_tensor(
                            p_t[:], usb[:], vrep[:, h * KD:(h + 1) * KD],
                            OP.mult)
                        bsl = b_state[:, t * KN:(t + 1) * KN]
                        if r == 1:
                            nc.vector.tensor_reduce(
                                bsl, p_t[:].rearrange("p (k d) -> p k d", k=KN),
                                AX.X, OP.add)
                        else:
                            a2 = small.tile([128, KN], F32, name="a2")
                            nc.vector.tensor_reduce(
                                a2[:], p_t[:].rearrange("p (k d) -> p k d", k=KN),
                                AX.X, OP.add)
                            nc.vector.tensor_tensor(bsl, bsl, a2[:], OP.add)
                        # c = softmax_k(b)  (no max-sub; |b| < ~16)
                        e_t = small.tile([128, KN], F32, name="e_t")
                        nc.scalar.activation(e_t[:], bsl, ACTF.Exp)
                        rs = small.tile([128, 1], F32, name="rs")
                        nc.vector.tensor_reduce(rs[:], e_t[:], AX.X, OP.add)
                        rc = small.tile([128, 1], F32, name="rc")
                        nc.vector.reciprocal(rc[:], rs[:])
                        cbf = small.tile([128, KN], BF16, name="cbf")
                        nc.vector.tensor_scalar_mul(cbf[:], e_t[:], rc[:])
                        cu = cup.tile([128, KD], BF16, name="cu")
                        nc.gpsimd.tensor_tensor(
                            cu[:], usb[:],
                            cbf[:].unsqueeze(2).broadcast_to((128, KN, D)),
                            OP.mult)
                        for j in range(4):
                            nc.tensor.matmul(
                                s_ps[:, j * 512:(j + 1) * 512],
                                bst(h), cu[:, j * 512:(j + 1) * 512],
                                start=(t == 0), stop=(t == 2 * NIB - 1))
                nc.scalar.copy(s_sb[:], s_ps[:])
                if no_cc:
                    if r == ROUTINGS - 1:
                        nc.sync.dma_start(v_out_d.ap(), s_sb[:])
                else:
                    nc.sync.dma_start(cc_in[r].ap(), s_sb[:])
                    allreduce(r)
                    squash_and_bcast(r, 1.0, last=(r == ROUTINGS - 1))

    nc.compile()
    return nc


def _prep_inputs(x, W):
    """Per-core host-side input prep. x [B, 2048, 16] f32, W [2048,64,32,16]."""
    bf = ml_dtypes.bfloat16
    in_maps = []
    for c in range(N_CORES):
        sl = slice(c * NI, (c + 1) * NI)
        Wc = W[sl]                                   # [256, 64, 32, 16]
        wr = np.ascontiguousarray(
            Wc.transpose(0, 3, 1, 2).reshape(NIB, 128, KD)).astype(bf)
        xc = x[:, sl, :]                             # [32, 256, 16]
        xt = np.ascontiguousarray(
            xc.transpose(1, 2, 0).reshape(NIB, 8, L, B)
              .reshape(NIB, 128, B)).astype(bf)
        # SX[(ib h), i8*16+l, i8*16+bl] = x[h*16+bl, ib*8+i8, l]
        sx = np.zeros((2 * NIB, 128, 128), np.float32)
        t5 = xc.reshape(2, 16, NIB, 8, L).transpose(2, 0, 3, 4, 1)
        # t5: [ib, h, i8, l, bl]
        for i8 in range(8):
            sx.reshape(NIB, 2, 128, 128)[
                :, :, i8 * 16:(i8 + 1) * 16, i8 * 16:(i8 + 1) * 16] = \
                t5[:, :, i8]
        sx = sx.astype(bf)
        bsm = np.zeros((2, 128, B), np.float32)
        for h in range(2):
            for i8 in range(8):
                for bl in range(16):
                    bsm[h, i8 * 16 + bl, h * 16 + bl] = 1.0
        bsm = bsm.astype(bf)
        in_maps.append({"wr": wr, "sx": sx, "xt": xt, "bs": bsm})
    return in_maps


def kernel(x, W):
    x = np.asarray(x, dtype=np.float32)
    W = np.asarray(W, dtype=np.float32)
    if "nc" not in _CACHE:
        _CACHE["nc"] = _build_nc()
    nc = _CACHE["nc"]
    in_maps = _prep_inputs(x, W)
    import time as _time
    t0 = _time.time()
    res = bass_utils.run_bass_kernel_spmd(
        nc, in_maps, core_ids=list(range(N_CORES)))
    _CACHE["exec_wall_ns"] = int((_time.time() - t0) * 1e9)
    v = res.results[0]["v_out"].reshape(B, KN, D).astype(np.float32)
    return v
